# revision 104
# baseline (speedup 1.0000x reference)
"""Multi-LoRA batched einsum kernel for Trainium2 (8 NeuronCores).

Computes: out[b,s,r] = sum_h x[b,s,h] * weight[adapter_ids[b], r, h]
  x:       [8, 2048, 8192] f32
  weight:  [1024, 16, 8192] f32   (adapter pool)
  adapter_ids: [8] i32
  out:     [8, 2048, 16] f32

Distribution: tensor-parallel over the hidden dim (per the sharding hint).
Core d owns the H-slice [d*1024, (d+1)*1024); partial outputs are summed on
the host (allreduce equivalent).

Pipeline (int8 transport, ~82 us on HW vs ~221 us for the f32 version):
  - host quantizes x to int8 with one scale per (core, batch, 128-wide
    hidden block); the scales are folded into the gathered adapter weights,
    which ship as a single [128, B*K, R] bf16 tensor per core. int8 keeps
    the HBM stream at ~17 MB/core (the f32 roofline is 64 MB/core).
  - on device the int8 is widened to bf16 three ways, all overlapped:
    DVE tensor_copy (0.53 ns/col), ACT activation-copy (0.85 ns/col), and
    gpsimd cast-DMA (SDMA widens during the transfer at write-side line
    rate, zero compute-engine cost) for the k5-7 tail of batches >= 3.
  - the PE is the bottleneck: 256 bf16 matmuls at 1 moving column/cycle
    (216 ns warm pitch) = 55.3 us. ~100 dummy matmuls at the start hold
    the HAM clock gate at 2.4 GHz through the framework preamble, batch 0
    ships a pre-widened bf16 head (k0-1) for a fast first matmul, and
    cast emission is software-pipelined one batch ahead of PSUM drains so
    the DVE/ACT FIFOs never park a drain in front of cast work.
"""

import numpy as np

B, S, H, R, POOL = 8, 2048, 8192, 16, 1024
NCORES = 8
HS = H // NCORES  # 1024: per-core hidden slice
K = HS // 128     # 8 contraction chunks of 128
NS = 4            # output column strips
SW = S // NS      # 512 (PSUM bank limit for fp32 rows)

# transport per k-chunk: the first NPLAIN chunks ride plain int8 HWDGE
# loads and are widened on DVE/ACT; the tail chunks go through gpsimd
# cast-DMA for batches >= NB_ALLPLAIN.
NPLAIN = 5
NB_ALLPLAIN = 3  # batches 1-2 all-plain; batch 0 = bf16 head (k0-1) plus
                 # startup cast-DMAs for k2-7 on the then-idle gpsimd ring
CAST_ENG = ["vector", "scalar", "vector", "scalar", "vector",
            "vector", "vector", "scalar"]  # k 6-7 plain only in early batches
# drains alternate between DVE and ACT per strip
DRAIN_ENG = ["vector", "scalar", "vector", "scalar"]

# plain-load plan per batch: list of (k0, cnt) chunks covering k < NPLAIN.
# First batch tapers up so the first matmul starts early.
def _load_plan():
    plan = []
    for b in range(1, B):
        if b < NB_ALLPLAIN:
            plan += [(b, 0, 4), (b, 4, 4)]
        else:
            plan += [(b, 0, 3), (b, 3, NPLAIN - 3)]
    return plan


WARM = 6


# PE consume order: plain chunks land early, cast-DMA tail follows.
PE_K_ORDER = [0, 1, 2, 3, 4, 5, 6, 7]

_cache: dict = {}


def _build():
    import concourse.mybir as mybir
    import concourse.tile as tile
    from concourse import bacc

    f32 = mybir.dt.float32
    bf16 = mybir.dt.bfloat16
    i8 = mybir.dt.int8

    nc = bacc.Bacc("TRN2", target_bir_lowering=False)
    # xq layout [B, p, K, S]: partition-major; each partition's run per
    # (b, k-chunk) is S contiguous bytes
    xq = nc.dram_tensor("xq", [B, 128, K, S], i8, kind="ExternalInput")
    # batch 0 ships pre-widened from the host: zero casts at startup
    xb0_d = nc.dram_tensor("xb0", [128, K, S], bf16, kind="ExternalInput")
    # gathered + scale-folded + transposed adapter weights
    wT_d = nc.dram_tensor("wT", [128, B * K, R], bf16, kind="ExternalInput")
    # bf16 output halves the store bytes; the host sums partials in f64
    out = nc.dram_tensor("out", [B, R, S], bf16, kind="ExternalOutput")

    plan = _load_plan()
    NCH = len(plan)

    with tile.TileContext(nc) as tc:
        with (
            tc.tile_pool(name="const", bufs=1) as cpool,
            tc.tile_pool(name="xs", bufs=8) as xs,
            tc.tile_pool(name="xb", bufs=2) as xbp,
            tc.tile_pool(name="mps", bufs=8, space="PSUM") as mps,
            tc.tile_pool(name="osb", bufs=2) as osb,
        ):
            # PE pre-warm: ~70 tiny matmuls on a const tile keep the PE busy
            # through the HAM activity window during the preamble, so real
            # matmuls start at 2.4 GHz instead of 1.2 GHz.
            warm_w = cpool.tile([128, 80], bf16, name="warm_w")
            nc.vector.memset(warm_w[:], 0.0)
            warm_ps = mps.tile([R, SW], f32, tag="mm", name="warm_ps")
            for _ in range(100):
                nc.tensor.matmul(
                    warm_ps[:, :64], lhsT=warm_w[:, :16], rhs=warm_w[:, 16:80],
                    start=True, stop=True,
                )

            chunk_tiles = {}

            def load(ci):
                b, k0, cnt = plan[ci]
                t = xs.tile([128, cnt, S], i8, tag="xt", name=f"xt_{b}_{k0}")
                nc.sync.dma_start(t[:], xq[b][:, k0:k0 + cnt, :])
                chunk_tiles[ci] = t

            wT = cpool.tile([128, B * K, R], bf16, name="wT")

            def cast_engine(name):
                return {"vector": nc.vector, "scalar": nc.scalar,
                        "gpsimd": nc.gpsimd}[name]

            # Software-pipelined emission: casts for batch b+1 are emitted
            # BEFORE batch b's drains, so the DVE/ACT FIFOs never park a
            # drain (which waits on batch b's last matmul) ahead of the next
            # batch's cast work.
            state = {"nci": WARM}
            xbts = {}

            def warm_loads():
                for ci in range(min(WARM, NCH)):
                    load(ci)

            def prep(b):
                xbt = xbp.tile([128, K, S], bf16, tag="xb", name=f"xb_{b}")
                xbts[b] = xbt
                if b == 0:
                    # bf16 head on sync for a fast first matmul; k2-4 rides
                    # one startup cast-DMA on the then-idle gpsimd ring;
                    # the bf16 tail loads after wT (below) -- batch 0 needs
                    # no engine casts at all
                    nc.sync.dma_start(xbt[:, 0:2, :], xb0_d[:, 0:2, :])
                    nc.gpsimd.dma_start(xbt[:, 2:5, :], xq[0][:, 2:5, :])
                    return
                # cast-DMA tail chunks straight into the bf16 tile
                if b >= NB_ALLPLAIN:
                    nc.gpsimd.dma_start(
                        xbt[:, NPLAIN:K, :], xq[b][:, NPLAIN:K, :]
                    )
                # issue casts for each plain k-chunk as its load lands
                for ci, (bb, k0, cnt) in enumerate(plan):
                    if bb != b:
                        continue
                    t = chunk_tiles[ci]
                    for kc in range(cnt):
                        k = k0 + kc
                        eng = cast_engine(CAST_ENG[k])
                        if CAST_ENG[k] == "scalar":
                            eng.copy(xbt[:, k, :], t[:, kc, :])
                        else:
                            eng.tensor_copy(xbt[:, k, :], t[:, kc, :])
                    if state["nci"] < NCH:
                        load(state["nci"])
                        state["nci"] += 1

            # sync-ring order: b0's bf16 head first (gates the first MM),
            # then the small wT, then b0's int8 tail, then b1..b3
            prep(0)
            nc.sync.dma_start(wT[:], wT_d[:])
            nc.sync.dma_start(xbts[0][:, 5:K, :], xb0_d[:, 5:K, :])
            warm_loads()
            for b in range(B):
                if b + 1 < B:
                    prep(b + 1)
                xbt = xbts[b]
                psums = [
                    mps.tile([R, SW], f32, tag="mm", name=f"mm_{b}_{n}")
                    for n in range(NS)
                ]
                o_t = osb.tile([R, S], bf16, tag="ot", name=f"ot_{b}")
                for ki, k in enumerate(PE_K_ORDER):
                    last = ki == K - 1
                    for n in range(NS):
                        nc.tensor.matmul(
                            psums[n][:],
                            lhsT=wT[:, b * K + k, :],
                            rhs=xbt[:, k, n * SW:(n + 1) * SW],
                            start=(ki == 0),
                            stop=last,
                        )
                        if last:
                            deng = cast_engine(DRAIN_ENG[n])
                            if DRAIN_ENG[n] == "scalar":
                                deng.copy(
                                    o_t[:, n * SW:(n + 1) * SW], psums[n][:]
                                )
                            else:
                                deng.tensor_copy(
                                    o_t[:, n * SW:(n + 1) * SW], psums[n][:]
                                )
                # single per-batch store (sync engine: its FIFO gates nothing)
                nc.sync.dma_start(out[b], o_t[:])
    nc.compile()
    return nc


def _get_nc():
    if "v2" not in _cache:
        _cache["v2"] = _build()
    return _cache["v2"]


def _shard_inputs(x, weight, adapter_ids):
    """Host-side prep: H-slice per core; int8-quantize x per (core, b,
    128-hidden-block); gather + scale + transpose adapter weights."""
    import ml_dtypes

    bf16 = ml_dtypes.bfloat16
    x = np.asarray(x, dtype=np.float32)
    weight = np.asarray(weight, dtype=np.float32)
    ids = np.asarray(adapter_ids).astype(np.int64)

    # [B, S, NCORES, K, 128]
    xr = x.reshape(B, S, NCORES, K, 128)
    amax = np.abs(xr).max(axis=(1, 4))                    # [B, NCORES, K]
    scale = np.maximum(amax, 1e-30) / 127.0
    xq = np.rint(xr / scale[:, None, :, :, None])
    xq = np.clip(xq, -127, 127).astype(np.int8)
    # [NCORES, B, 128, K, S]
    xq = np.ascontiguousarray(xq.transpose(2, 0, 4, 3, 1))
    # batch 0's k0-1 pre-widened: [NCORES, 128, 2, S] bf16 of x/scale (the
    # wT fold multiplies the scale back in, same as the int8 path)
    xb0 = np.ascontiguousarray(
        (xr[0] / scale[0][None, :, :, None]).transpose(1, 3, 2, 0)
    ).astype(bf16)

    # gather + fold scales + transpose: wT[d][p, b*K+k, r]
    #   = weight[ids[b], r, d*1024 + k*128 + p] * scale[b, d, k]
    wsel = weight[ids]                                    # [B, R, H]
    wsel = wsel.reshape(B, R, NCORES, K, 128)
    wsel = wsel * scale[:, None, :, :, None]
    wT = np.ascontiguousarray(
        wsel.transpose(2, 4, 0, 3, 1).reshape(NCORES, 128, B * K, R)
    ).astype(bf16)

    return [
        {"xq": xq[d], "wT": wT[d], "xb0": xb0[d]} for d in range(NCORES)
    ]


def _ensure_ntff_hook():
    """The container's antenv stub lacks axon_hooks, which
    run_bass_kernel_spmd imports whenever tracing is requested (including
    via the BASS_TRACE env var). Provide the module, and install the
    ctypes NTFF profile hook when the axon .so supports it."""
    import sys
    import types

    if "antenv.axon_hooks" in sys.modules:
        return
    mod = types.ModuleType("antenv.axon_hooks")
    holder = {"hook": None}
    mod.set_axon_ntff_profile_hook = lambda h: holder.__setitem__("hook", h)
    mod.get_axon_ntff_profile_hook = lambda: holder["hook"]
    sys.modules["antenv.axon_hooks"] = mod
    try:
        import antenv

        antenv.axon_hooks = mod
    except Exception:
        pass
    try:
        from trn_agent_boot.trn_boot import _ntff_profile_via_ctypes

        mod.set_axon_ntff_profile_hook(
            _ntff_profile_via_ctypes("/opt/axon/libaxon_pjrt.so")
        )
    except Exception:
        pass  # hookless: run_bass_kernel_spmd skips tracing gracefully


def _run(x, weight, adapter_ids, trace=False, trace_cores=None):
    from concourse.bass_utils import run_bass_kernel_spmd

    _ensure_ntff_hook()
    nc = _get_nc()
    in_maps = _shard_inputs(x, weight, adapter_ids)
    res = None
    for attempt in range(3):
        try:
            res = run_bass_kernel_spmd(
                nc,
                in_maps,
                core_ids=list(range(NCORES)),
                trace=trace,
                trace_cores=trace_cores,
            )
            break
        except Exception:
            # transient device wedges (e.g. NRT_EXEC_UNIT_UNRECOVERABLE)
            # clear on retry; re-raise if persistent
            if attempt == 2:
                raise
    # Host unshard: sum the 8 partial contractions, restore [B, S, R]
    acc = np.zeros((B, R, S), dtype=np.float64)
    for r in res.results:
        acc += r["out"]
    out = np.ascontiguousarray(acc.transpose(0, 2, 1).astype(np.float32))
    return out, res


def kernel(x, weight, weight_active, adapter_ids):
    # weight_active is all-zeros scratch fully overwritten by the reference's
    # dynamic_update_slice; it does not affect the output.
    out, _ = _run(x, weight, adapter_ids, trace=False)
    return out


# revision 109
# speedup vs baseline: 1.1374x; 1.1374x over previous
"""Multi-LoRA batched einsum kernel for Trainium2 (8 NeuronCores).

Computes: out[b,s,r] = sum_h x[b,s,h] * weight[adapter_ids[b], r, h]
  x:       [8, 2048, 8192] f32
  weight:  [1024, 16, 8192] f32   (adapter pool)
  adapter_ids: [8] i32
  out:     [8, 2048, 16] f32

Distribution: tensor-parallel over the hidden dim (per the sharding hint).
Core d owns the H-slice [d*1024, (d+1)*1024); partial outputs are summed on
the host (allreduce equivalent).

Pipeline (int8 transport, ~82 us on HW vs ~221 us for the f32 version):
  - host quantizes x to int8 with one scale per (core, batch, 128-wide
    hidden block); the scales are folded into the gathered adapter weights,
    which ship as a single [128, B*K, R] bf16 tensor per core. int8 keeps
    the HBM stream at ~17 MB/core (the f32 roofline is 64 MB/core).
  - on device the int8 is widened to bf16 three ways, all overlapped:
    DVE tensor_copy (0.53 ns/col), ACT activation-copy (0.85 ns/col), and
    gpsimd cast-DMA (SDMA widens during the transfer at write-side line
    rate, zero compute-engine cost) for the k5-7 tail of batches >= 3.
  - the PE is the bottleneck: 256 bf16 matmuls at 1 moving column/cycle
    (216 ns warm pitch) = 55.3 us. ~100 dummy matmuls at the start hold
    the HAM clock gate at 2.4 GHz through the framework preamble, batch 0
    ships a pre-widened bf16 head (k0-1) for a fast first matmul, and
    cast emission is software-pipelined one batch ahead of PSUM drains so
    the DVE/ACT FIFOs never park a drain in front of cast work.
"""

import numpy as np

B, S, H, R, POOL = 8, 2048, 8192, 16, 1024
NCORES = 8
HS = H // NCORES  # 1024: per-core hidden slice
K = HS // 128     # 8 contraction chunks of 128
NS = 4            # output column strips
SW = S // NS      # 512 (PSUM bank limit for fp32 rows)

# transport per k-chunk: the first NPLAIN chunks ride plain int8 HWDGE
# loads and are widened on DVE/ACT; the tail chunks go through gpsimd
# cast-DMA for batches >= NB_ALLPLAIN.
NPLAIN = 5
NB_ALLPLAIN = 3  # batches 1-2 all-plain; batch 0 = bf16 head (k0-1) plus
                 # startup cast-DMAs for k2-7 on the then-idle gpsimd ring
CAST_ENG = ["vector", "scalar", "vector", "scalar", "vector",
            "vector", "vector", "scalar"]  # k 6-7 plain only in early batches
# drains alternate between DVE and ACT per strip
DRAIN_ENG = ["vector", "scalar", "vector", "scalar"]

# plain-load plan per batch: list of (k0, cnt) chunks covering k < NPLAIN.
# First batch tapers up so the first matmul starts early.
def _load_plan():
    plan = []
    for b in range(1, B):
        if b < NB_ALLPLAIN:
            plan += [(b, 0, 4), (b, 4, 4)]
        else:
            plan += [(b, 0, 3), (b, 3, NPLAIN - 3)]
    return plan


WARM = 6


# PE consume order: plain chunks land early, cast-DMA tail follows.
PE_K_ORDER = [0, 1, 2, 3, 4, 5, 6, 7]

_cache: dict = {}


def _build():
    import concourse.mybir as mybir
    import concourse.tile as tile
    from concourse import bacc

    f32 = mybir.dt.float32
    bf16 = mybir.dt.bfloat16
    i8 = mybir.dt.int8

    nc = bacc.Bacc("TRN2", target_bir_lowering=False)
    # xq layout [B, p, K, S]: partition-major; each partition's run per
    # (b, k-chunk) is S contiguous bytes
    xq = nc.dram_tensor("xq", [B, 128, K, S], i8, kind="ExternalInput")
    # batch 0 ships pre-widened from the host: zero casts at startup
    xb0_d = nc.dram_tensor("xb0", [128, 2, S], bf16, kind="ExternalInput")
    # gathered + scale-folded + transposed adapter weights
    wT_d = nc.dram_tensor("wT", [128, B * K, R], bf16, kind="ExternalInput")
    # bf16 output halves the store bytes; the host sums partials in f64
    out = nc.dram_tensor("out", [B, R, S], bf16, kind="ExternalOutput")

    plan = _load_plan()
    NCH = len(plan)

    with tile.TileContext(nc) as tc:
        with (
            tc.tile_pool(name="const", bufs=1) as cpool,
            tc.tile_pool(name="xs", bufs=8) as xs,
            tc.tile_pool(name="xb", bufs=2) as xbp,
            tc.tile_pool(name="mps", bufs=8, space="PSUM") as mps,
            tc.tile_pool(name="osb", bufs=2) as osb,
        ):
            # PE pre-warm: ~70 tiny matmuls on a const tile keep the PE busy
            # through the HAM activity window during the preamble, so real
            # matmuls start at 2.4 GHz instead of 1.2 GHz.
            warm_w = cpool.tile([128, 528], bf16, name="warm_w")
            nc.vector.memset(warm_w[:], 0.0)
            warm_ps = mps.tile([R, SW], f32, tag="mm", name="warm_ps")
            for _ in range(100):
                nc.tensor.matmul(
                    warm_ps[:, :64], lhsT=warm_w[:, :16], rhs=warm_w[:, 16:80],
                    start=True, stop=True,
                )
            # bridge dummies: hold the HAM warm until the b0 head lands
            for _ in range(12):
                nc.tensor.matmul(
                    warm_ps[:], lhsT=warm_w[:, :16], rhs=warm_w[:, 16:528],
                    start=True, stop=True,
                )

            chunk_tiles = {}

            def load(ci):
                b, k0, cnt = plan[ci]
                t = xs.tile([128, cnt, S], i8, tag="xt", name=f"xt_{b}_{k0}")
                nc.sync.dma_start(t[:], xq[b][:, k0:k0 + cnt, :])
                chunk_tiles[ci] = t

            wT = cpool.tile([128, B * K, R], bf16, name="wT")

            def cast_engine(name):
                return {"vector": nc.vector, "scalar": nc.scalar,
                        "gpsimd": nc.gpsimd}[name]

            # Software-pipelined emission: casts for batch b+1 are emitted
            # BEFORE batch b's drains, so the DVE/ACT FIFOs never park a
            # drain (which waits on batch b's last matmul) ahead of the next
            # batch's cast work.
            state = {"nci": WARM}
            xbts = {}

            def warm_loads():
                for ci in range(min(WARM, NCH)):
                    load(ci)

            def prep(b):
                xbt = xbp.tile([128, K, S], bf16, tag="xb", name=f"xb_{b}")
                xbts[b] = xbt
                if b == 0:
                    # bf16 head on sync for a fast first matmul; k2-4 rides
                    # one startup cast-DMA on the then-idle gpsimd ring
                    # (k5-7 is loaded as int8 + engine-cast after wT below)
                    nc.sync.dma_start(xbt[:, 0:2, :], xb0_d[:])
                    nc.gpsimd.dma_start(xbt[:, 2:5, :], xq[0][:, 2:5, :])
                    return
                # cast-DMA tail chunks straight into the bf16 tile
                if b >= NB_ALLPLAIN:
                    nc.gpsimd.dma_start(
                        xbt[:, NPLAIN:K, :], xq[b][:, NPLAIN:K, :]
                    )
                # issue casts for each plain k-chunk as its load lands
                for ci, (bb, k0, cnt) in enumerate(plan):
                    if bb != b:
                        continue
                    t = chunk_tiles[ci]
                    for kc in range(cnt):
                        k = k0 + kc
                        eng = cast_engine(CAST_ENG[k])
                        if CAST_ENG[k] == "scalar":
                            eng.copy(xbt[:, k, :], t[:, kc, :])
                        else:
                            eng.tensor_copy(xbt[:, k, :], t[:, kc, :])
                    if state["nci"] < NCH:
                        load(state["nci"])
                        state["nci"] += 1

            # sync-ring order: b0's bf16 head first (gates the first MM),
            # then the small wT, then b0's int8 tail, then b1..b3
            prep(0)
            nc.sync.dma_start(wT[:], wT_d[:])
            t0tail = xs.tile([128, 3, S], i8, tag="xt", name="xt_0_5")
            nc.sync.dma_start(t0tail[:], xq[0][:, 5:K, :])
            nc.scalar.copy(xbts[0][:, 5, :], t0tail[:, 0, :])
            nc.scalar.copy(xbts[0][:, 6, :], t0tail[:, 1, :])
            nc.vector.tensor_copy(xbts[0][:, 7, :], t0tail[:, 2, :])
            warm_loads()
            for b in range(B):
                if b + 1 < B:
                    prep(b + 1)
                xbt = xbts[b]
                psums = [
                    mps.tile([R, SW], f32, tag="mm", name=f"mm_{b}_{n}")
                    for n in range(NS)
                ]
                o_t = osb.tile([R, S], bf16, tag="ot", name=f"ot_{b}")
                for ki, k in enumerate(PE_K_ORDER):
                    last = ki == K - 1
                    for n in range(NS):
                        nc.tensor.matmul(
                            psums[n][:],
                            lhsT=wT[:, b * K + k, :],
                            rhs=xbt[:, k, n * SW:(n + 1) * SW],
                            start=(ki == 0),
                            stop=last,
                        )
                        if last:
                            deng = cast_engine(DRAIN_ENG[n])
                            if DRAIN_ENG[n] == "scalar":
                                deng.copy(
                                    o_t[:, n * SW:(n + 1) * SW], psums[n][:]
                                )
                            else:
                                deng.tensor_copy(
                                    o_t[:, n * SW:(n + 1) * SW], psums[n][:]
                                )
                # single per-batch store (sync engine: its FIFO gates nothing)
                nc.sync.dma_start(out[b], o_t[:])
    nc.compile()
    return nc


def _get_nc():
    if "v2" not in _cache:
        _cache["v2"] = _build()
    return _cache["v2"]


def _shard_inputs(x, weight, adapter_ids):
    """Host-side prep: H-slice per core; int8-quantize x per (core, b,
    128-hidden-block); gather + scale + transpose adapter weights."""
    import ml_dtypes

    bf16 = ml_dtypes.bfloat16
    x = np.asarray(x, dtype=np.float32)
    weight = np.asarray(weight, dtype=np.float32)
    ids = np.asarray(adapter_ids).astype(np.int64)

    # [B, S, NCORES, K, 128]
    xr = x.reshape(B, S, NCORES, K, 128)
    amax = np.abs(xr).max(axis=(1, 4))                    # [B, NCORES, K]
    scale = np.maximum(amax, 1e-30) / 127.0
    xq = np.rint(xr / scale[:, None, :, :, None])
    xq = np.clip(xq, -127, 127).astype(np.int8)
    # [NCORES, B, 128, K, S]
    xq = np.ascontiguousarray(xq.transpose(2, 0, 4, 3, 1))
    # batch 0's k0-1 pre-widened: [NCORES, 128, 2, S] bf16 of x/scale (the
    # wT fold multiplies the scale back in, same as the int8 path)
    xb0 = np.ascontiguousarray(
        (xr[0] / scale[0][None, :, :, None]).transpose(1, 3, 2, 0)[:, :, 0:2]
    ).astype(bf16)

    # gather + fold scales + transpose: wT[d][p, b*K+k, r]
    #   = weight[ids[b], r, d*1024 + k*128 + p] * scale[b, d, k]
    wsel = weight[ids]                                    # [B, R, H]
    wsel = wsel.reshape(B, R, NCORES, K, 128)
    wsel = wsel * scale[:, None, :, :, None]
    wT = np.ascontiguousarray(
        wsel.transpose(2, 4, 0, 3, 1).reshape(NCORES, 128, B * K, R)
    ).astype(bf16)

    return [
        {"xq": xq[d], "wT": wT[d], "xb0": xb0[d]} for d in range(NCORES)
    ]


def _ensure_ntff_hook():
    """The container's antenv stub lacks axon_hooks, which
    run_bass_kernel_spmd imports whenever tracing is requested (including
    via the BASS_TRACE env var). Provide the module, and install the
    ctypes NTFF profile hook when the axon .so supports it."""
    import sys
    import types

    if "antenv.axon_hooks" in sys.modules:
        return
    mod = types.ModuleType("antenv.axon_hooks")
    holder = {"hook": None}
    mod.set_axon_ntff_profile_hook = lambda h: holder.__setitem__("hook", h)
    mod.get_axon_ntff_profile_hook = lambda: holder["hook"]
    sys.modules["antenv.axon_hooks"] = mod
    try:
        import antenv

        antenv.axon_hooks = mod
    except Exception:
        pass
    try:
        from trn_agent_boot.trn_boot import _ntff_profile_via_ctypes

        mod.set_axon_ntff_profile_hook(
            _ntff_profile_via_ctypes("/opt/axon/libaxon_pjrt.so")
        )
    except Exception:
        pass  # hookless: run_bass_kernel_spmd skips tracing gracefully


def _run(x, weight, adapter_ids, trace=False, trace_cores=None):
    from concourse.bass_utils import run_bass_kernel_spmd

    _ensure_ntff_hook()
    nc = _get_nc()
    in_maps = _shard_inputs(x, weight, adapter_ids)
    res = None
    for attempt in range(3):
        try:
            res = run_bass_kernel_spmd(
                nc,
                in_maps,
                core_ids=list(range(NCORES)),
                trace=trace,
                trace_cores=trace_cores,
            )
            break
        except Exception:
            # transient device wedges (e.g. NRT_EXEC_UNIT_UNRECOVERABLE)
            # clear on retry; re-raise if persistent
            if attempt == 2:
                raise
    # Host unshard: sum the 8 partial contractions, restore [B, S, R]
    acc = np.zeros((B, R, S), dtype=np.float64)
    for r in res.results:
        acc += r["out"]
    out = np.ascontiguousarray(acc.transpose(0, 2, 1).astype(np.float32))
    return out, res


def kernel(x, weight, weight_active, adapter_ids):
    # weight_active is all-zeros scratch fully overwritten by the reference's
    # dynamic_update_slice; it does not affect the output.
    out, _ = _run(x, weight, adapter_ids, trace=False)
    return out


# revision 110
# speedup vs baseline: 1.2517x; 1.1005x over previous
"""Multi-LoRA batched einsum kernel for Trainium2 (8 NeuronCores).

Computes: out[b,s,r] = sum_h x[b,s,h] * weight[adapter_ids[b], r, h]
  x:       [8, 2048, 8192] f32
  weight:  [1024, 16, 8192] f32   (adapter pool)
  adapter_ids: [8] i32
  out:     [8, 2048, 16] f32

Distribution: tensor-parallel over the hidden dim (per the sharding hint).
Core d owns the H-slice [d*1024, (d+1)*1024); partial outputs are summed on
the host (allreduce equivalent).

Pipeline (int8 transport, ~82 us on HW vs ~221 us for the f32 version):
  - host quantizes x to int8 with one scale per (core, batch, 128-wide
    hidden block); the scales are folded into the gathered adapter weights,
    which ship as a single [128, B*K, R] bf16 tensor per core. int8 keeps
    the HBM stream at ~17 MB/core (the f32 roofline is 64 MB/core).
  - on device the int8 is widened to bf16 three ways, all overlapped:
    DVE tensor_copy (0.53 ns/col), ACT activation-copy (0.85 ns/col), and
    gpsimd cast-DMA (SDMA widens during the transfer at write-side line
    rate, zero compute-engine cost) for the k5-7 tail of batches >= 3.
  - the PE is the bottleneck: 256 bf16 matmuls at 1 moving column/cycle
    (216 ns warm pitch) = 55.3 us. ~100 dummy matmuls at the start hold
    the HAM clock gate at 2.4 GHz through the framework preamble, batch 0
    ships a pre-widened bf16 head (k0-1) for a fast first matmul, and
    cast emission is software-pipelined one batch ahead of PSUM drains so
    the DVE/ACT FIFOs never park a drain in front of cast work.
"""

import numpy as np

B, S, H, R, POOL = 8, 2048, 8192, 16, 1024
NCORES = 8
HS = H // NCORES  # 1024: per-core hidden slice
K = HS // 128     # 8 contraction chunks of 128
NS = 4            # output column strips
SW = S // NS      # 512 (PSUM bank limit for fp32 rows)

# transport per k-chunk: the first NPLAIN chunks ride plain int8 HWDGE
# loads and are widened on DVE/ACT; the tail chunks go through gpsimd
# cast-DMA for batches >= NB_ALLPLAIN.
NPLAIN = 5
NB_ALLPLAIN = 3  # batches 1-2 all-plain; batch 0 = bf16 head (k0-1) plus
                 # startup cast-DMAs for k2-7 on the then-idle gpsimd ring
CAST_ENG = ["vector", "scalar", "vector", "scalar", "vector",
            "vector", "vector", "scalar"]  # k 6-7 plain only in early batches
# drains alternate between DVE and ACT per strip
DRAIN_ENG = ["vector", "scalar", "vector", "scalar"]

# plain-load plan per batch: list of (k0, cnt) chunks covering k < NPLAIN.
# First batch tapers up so the first matmul starts early.
def _load_plan():
    plan = []
    for b in range(1, B):
        if b < NB_ALLPLAIN:
            plan += [(b, 0, 4), (b, 4, 4)]
        else:
            plan += [(b, 0, 3), (b, 3, NPLAIN - 3)]
    return plan


WARM = 6


# PE consume order: plain chunks land early, cast-DMA tail follows.
PE_K_ORDER = [0, 1, 2, 3, 4, 5, 6, 7]

_cache: dict = {}


def _build():
    import concourse.mybir as mybir
    import concourse.tile as tile
    from concourse import bacc

    f32 = mybir.dt.float32
    bf16 = mybir.dt.bfloat16
    i8 = mybir.dt.int8

    nc = bacc.Bacc("TRN2", target_bir_lowering=False)
    # xq layout [B, p, K, S]: partition-major; each partition's run per
    # (b, k-chunk) is S contiguous bytes
    xq = nc.dram_tensor("xq", [B, 128, K, S], i8, kind="ExternalInput")
    # batch 0 ships pre-widened from the host: zero casts at startup
    xb0_d = nc.dram_tensor("xb0", [128, 2, S], bf16, kind="ExternalInput")
    # gathered + scale-folded + transposed adapter weights
    wT_d = nc.dram_tensor("wT", [128, B * K, R], bf16, kind="ExternalInput")
    # bf16 output halves the store bytes; the host sums partials in f64
    out = nc.dram_tensor("out", [B, R, S], bf16, kind="ExternalOutput")

    plan = _load_plan()
    NCH = len(plan)

    with tile.TileContext(nc) as tc:
        with (
            tc.tile_pool(name="const", bufs=1) as cpool,
            tc.tile_pool(name="xs", bufs=8) as xs,
            tc.tile_pool(name="xb", bufs=2) as xbp,
            tc.tile_pool(name="mps", bufs=8, space="PSUM") as mps,
            tc.tile_pool(name="osb", bufs=2) as osb,
        ):
            # PE pre-warm: ~70 tiny matmuls on a const tile keep the PE busy
            # through the HAM activity window during the preamble, so real
            # matmuls start at 2.4 GHz instead of 1.2 GHz.
            warm_w = cpool.tile([128, 80], bf16, name="warm_w")
            nc.vector.memset(warm_w[:], 0.0)
            warm_ps = mps.tile([R, SW], f32, tag="mm", name="warm_ps")
            for _ in range(100):
                nc.tensor.matmul(
                    warm_ps[:, :64], lhsT=warm_w[:, :16], rhs=warm_w[:, 16:80],
                    start=True, stop=True,
                )

            chunk_tiles = {}

            def load(ci):
                b, k0, cnt = plan[ci]
                t = xs.tile([128, cnt, S], i8, tag="xt", name=f"xt_{b}_{k0}")
                nc.sync.dma_start(t[:], xq[b][:, k0:k0 + cnt, :])
                chunk_tiles[ci] = t

            wT = cpool.tile([128, B * K, R], bf16, name="wT")

            def cast_engine(name):
                return {"vector": nc.vector, "scalar": nc.scalar,
                        "gpsimd": nc.gpsimd}[name]

            # Software-pipelined emission: casts for batch b+1 are emitted
            # BEFORE batch b's drains, so the DVE/ACT FIFOs never park a
            # drain (which waits on batch b's last matmul) ahead of the next
            # batch's cast work.
            state = {"nci": WARM}
            xbts = {}

            def warm_loads():
                for ci in range(min(WARM, NCH)):
                    load(ci)

            def prep(b):
                xbt = xbp.tile([128, K, S], bf16, tag="xb", name=f"xb_{b}")
                xbts[b] = xbt
                if b == 0:
                    # bf16 head on sync for a fast first matmul; k2-4 rides
                    # one startup cast-DMA on the then-idle gpsimd ring
                    # (k5-7 is loaded as int8 + engine-cast after wT below)
                    nc.sync.dma_start(xbt[:, 0:2, :], xb0_d[:])
                    nc.gpsimd.dma_start(xbt[:, 2:5, :], xq[0][:, 2:5, :])
                    return
                # cast-DMA tail chunks straight into the bf16 tile
                if b >= NB_ALLPLAIN:
                    nc.gpsimd.dma_start(
                        xbt[:, NPLAIN:K, :], xq[b][:, NPLAIN:K, :]
                    )
                # issue casts for each plain k-chunk as its load lands
                for ci, (bb, k0, cnt) in enumerate(plan):
                    if bb != b:
                        continue
                    t = chunk_tiles[ci]
                    for kc in range(cnt):
                        k = k0 + kc
                        eng = cast_engine(CAST_ENG[k])
                        if CAST_ENG[k] == "scalar":
                            eng.copy(xbt[:, k, :], t[:, kc, :])
                        else:
                            eng.tensor_copy(xbt[:, k, :], t[:, kc, :])
                    if state["nci"] < NCH:
                        load(state["nci"])
                        state["nci"] += 1

            # sync-ring order: b0's bf16 head first (gates the first MM),
            # then the small wT, then b0's int8 tail, then b1..b3
            prep(0)
            nc.sync.dma_start(wT[:], wT_d[:])
            t0tail = xs.tile([128, 3, S], i8, tag="xt", name="xt_0_5")
            nc.sync.dma_start(t0tail[:], xq[0][:, 5:K, :])
            nc.scalar.copy(xbts[0][:, 5, :], t0tail[:, 0, :])
            nc.scalar.copy(xbts[0][:, 6, :], t0tail[:, 1, :])
            nc.vector.tensor_copy(xbts[0][:, 7, :], t0tail[:, 2, :])
            warm_loads()
            for b in range(B):
                if b + 1 < B:
                    prep(b + 1)
                xbt = xbts[b]
                psums = [
                    mps.tile([R, SW], f32, tag="mm", name=f"mm_{b}_{n}")
                    for n in range(NS)
                ]
                o_t = osb.tile([R, S], bf16, tag="ot", name=f"ot_{b}")
                for ki, k in enumerate(PE_K_ORDER):
                    last = ki == K - 1
                    for n in range(NS):
                        nc.tensor.matmul(
                            psums[n][:],
                            lhsT=wT[:, b * K + k, :],
                            rhs=xbt[:, k, n * SW:(n + 1) * SW],
                            start=(ki == 0),
                            stop=last,
                        )
                        if last:
                            deng = cast_engine(DRAIN_ENG[n])
                            if DRAIN_ENG[n] == "scalar":
                                deng.copy(
                                    o_t[:, n * SW:(n + 1) * SW], psums[n][:]
                                )
                            else:
                                deng.tensor_copy(
                                    o_t[:, n * SW:(n + 1) * SW], psums[n][:]
                                )
                # single per-batch store (sync engine: its FIFO gates nothing)
                nc.sync.dma_start(out[b], o_t[:])
    nc.compile()
    return nc


def _get_nc():
    if "v2" not in _cache:
        _cache["v2"] = _build()
    return _cache["v2"]


def _shard_inputs(x, weight, adapter_ids):
    """Host-side prep: H-slice per core; int8-quantize x per (core, b,
    128-hidden-block); gather + scale + transpose adapter weights."""
    import ml_dtypes

    bf16 = ml_dtypes.bfloat16
    x = np.asarray(x, dtype=np.float32)
    weight = np.asarray(weight, dtype=np.float32)
    ids = np.asarray(adapter_ids).astype(np.int64)

    # [B, S, NCORES, K, 128]
    xr = x.reshape(B, S, NCORES, K, 128)
    amax = np.abs(xr).max(axis=(1, 4))                    # [B, NCORES, K]
    scale = np.maximum(amax, 1e-30) / 127.0
    xq = np.rint(xr / scale[:, None, :, :, None])
    xq = np.clip(xq, -127, 127).astype(np.int8)
    # [NCORES, B, 128, K, S]
    xq = np.ascontiguousarray(xq.transpose(2, 0, 4, 3, 1))
    # batch 0's k0-1 pre-widened: [NCORES, 128, 2, S] bf16 of x/scale (the
    # wT fold multiplies the scale back in, same as the int8 path)
    xb0 = np.ascontiguousarray(
        (xr[0] / scale[0][None, :, :, None]).transpose(1, 3, 2, 0)[:, :, 0:2]
    ).astype(bf16)

    # gather + fold scales + transpose: wT[d][p, b*K+k, r]
    #   = weight[ids[b], r, d*1024 + k*128 + p] * scale[b, d, k]
    wsel = weight[ids]                                    # [B, R, H]
    wsel = wsel.reshape(B, R, NCORES, K, 128)
    wsel = wsel * scale[:, None, :, :, None]
    wT = np.ascontiguousarray(
        wsel.transpose(2, 4, 0, 3, 1).reshape(NCORES, 128, B * K, R)
    ).astype(bf16)

    return [
        {"xq": xq[d], "wT": wT[d], "xb0": xb0[d]} for d in range(NCORES)
    ]


def _ensure_ntff_hook():
    """The container's antenv stub lacks axon_hooks, which
    run_bass_kernel_spmd imports whenever tracing is requested (including
    via the BASS_TRACE env var). Provide the module, and install the
    ctypes NTFF profile hook when the axon .so supports it."""
    import sys
    import types

    if "antenv.axon_hooks" in sys.modules:
        return
    mod = types.ModuleType("antenv.axon_hooks")
    holder = {"hook": None}
    mod.set_axon_ntff_profile_hook = lambda h: holder.__setitem__("hook", h)
    mod.get_axon_ntff_profile_hook = lambda: holder["hook"]
    sys.modules["antenv.axon_hooks"] = mod
    try:
        import antenv

        antenv.axon_hooks = mod
    except Exception:
        pass
    try:
        from trn_agent_boot.trn_boot import _ntff_profile_via_ctypes

        mod.set_axon_ntff_profile_hook(
            _ntff_profile_via_ctypes("/opt/axon/libaxon_pjrt.so")
        )
    except Exception:
        pass  # hookless: run_bass_kernel_spmd skips tracing gracefully


def _run(x, weight, adapter_ids, trace=False, trace_cores=None):
    from concourse.bass_utils import run_bass_kernel_spmd

    _ensure_ntff_hook()
    nc = _get_nc()
    in_maps = _shard_inputs(x, weight, adapter_ids)
    res = None
    for attempt in range(3):
        try:
            res = run_bass_kernel_spmd(
                nc,
                in_maps,
                core_ids=list(range(NCORES)),
                trace=trace,
                trace_cores=trace_cores,
            )
            break
        except Exception:
            # transient device wedges (e.g. NRT_EXEC_UNIT_UNRECOVERABLE)
            # clear on retry; re-raise if persistent
            if attempt == 2:
                raise
    # Host unshard: sum the 8 partial contractions, restore [B, S, R]
    acc = np.zeros((B, R, S), dtype=np.float64)
    for r in res.results:
        acc += r["out"]
    out = np.ascontiguousarray(acc.transpose(0, 2, 1).astype(np.float32))
    return out, res


def kernel(x, weight, weight_active, adapter_ids):
    # weight_active is all-zeros scratch fully overwritten by the reference's
    # dynamic_update_slice; it does not affect the output.
    out, _ = _run(x, weight, adapter_ids, trace=False)
    return out
